# revision 13
# baseline (speedup 1.0000x reference)
"""CSPN 3x3 per-pixel MAC kernel for Trainium2, 8-core data parallel.

out[b,0,h,w] = sum_{t in 0..8, t!=4} K[b,t,h,w] * xpad[b,h+t//3,w+t%3]
             + K[b,4,h,w] * input0[b,0,h,w]

Sharding: batch 16 -> 2 samples per core, pure data parallel.

Numerics (harness rel-err gate is 2e-2; this kernel measures 1.14e-2,
bit-exactly predicted by a host-side pipeline simulation): 6 of 9
kernel planes are float8_e3m4 (4-bit mantissa, range +-15.5 covers the
N(0,1) weights), the other 3 planes + x0 + input stay bf16; products
are bf16, accumulated in f32 PSUM; output bf16, upcast to f32 on host.
fp8 halves the dominant DMA stream again (24.0 -> 18.9 MB/core) at the
cost of the fp8-tap multiplies dropping out of the DVE 2x fast path
(1-byte operands run 1 elem/cycle) -- k=6 fp8 taps balances the DMA
roofline (~59us) against DVE busy (~57us).

Host-side repack: weights stored ROW-MAJOR-BY-OUTPUT-ROW: kern8
[SPC,H,6,W] fp8 (taps 0,2,3,5,6,8) and kern16 [SPC,H,4,W] bf16 (taps
1,4,7 + the x0 row) -- each band's weight data is 2 dma_starts of p
contiguous runs (sequential HBM walk).  A device-side transposed walk
over the natural [9,H,W] layout was measured ~10% slower (partition-
major descriptor order jumps 856KB between planes -> HBM scatter).

Work split across engines (measured: DVE compute and DMA traffic
contend heavily on this silicon -- spreading compute across engines
wins ~20% over DVE-only):
 - DVE: 9 elementwise products per band into a [p,9,W] bf16 tile.
 - PE:  accumulates the 9 product planes into f32 PSUM via
   identity-weight matmuls (out += I.T @ prod_t), 3 PSUM-bank-sized
   column chunks x 9 taps.
 - ACT: copies the f32 PSUM result to a bf16 SBUF tile (DMA cannot
   read PSUM) and issues the x/kern8 DMAs from its HWDGE ring.
 - SP:  issues the kern16 + out DMAs from its HWDGE ring (dedicated
   queues measured better than round-robin).

Negative results measured on this silicon/toolchain, do not retry:
 - GpSimd width-split co-execution: 30% slower, serializes vs DVE.
 - Deeper tile pools (bufs 3): consistently slower than bufs 2.
 - Splitting a band load into partition chunks: much worse (DMA-to-
   SBUF write throughput scales with destination partition count).
 - Partition-base-shifted compute operands: rejected by BIR verifier.
 - bf16 DVE writes to PSUM: rejected by compiler (matmul/memset only).
 - SBUF->SBUF partition-shift copies to single-read x: slower (SBUF
   DMA bytes are the contended resource; the 3x HBM re-read of x via
   one overlapping-AP dma_start is cheaper).
 - SWDGE (Pool-engine) DMA for the kern load: does not execute
   correctly inside hardware loops here.
 - float8_e4m3 weights: 2.7e-2 all-taps / 1.8e-2 4-taps -- over or
   too close to the 2e-2 gate; e3m4 halves the quantization error.
"""

import os
import sys

for _p in ("/opt/trn_rl_repo", "/root/.axon_site/_ro/trn_rl_repo"):
    if os.path.isdir(_p) and _p not in sys.path:
        sys.path.append(_p)

import ml_dtypes
import numpy as np

import concourse.bacc as bacc
import concourse.mybir as mybir
from concourse import bass_utils, tile
from concourse.ap import AP

KS = 3
BS, H, W = 16, 352, 1216
NCORES = 8
SPC = BS // NCORES          # samples per core = 2
HP, WP = H + 2, W + 2       # zero-padded dims: 354 x 1218
BF16 = mybir.dt.bfloat16
F8 = mybir.dt.float8e3
F32 = mybir.dt.float32
NP_BF16 = ml_dtypes.bfloat16
NP_F8 = ml_dtypes.float8_e3m4
MULT = mybir.AluOpType.mult

ROW_BANDS = [(0, 128), (128, 128), (256, 96)]
# PSUM-bank-sized (512 f32) column chunks of W for matmul accumulation
CHUNKS = [(0, 512), (512, 512), (1024, 192)]




def _build_nc(loop_reps=1):
    nc = bacc.Bacc(None)
    kern8 = nc.dram_tensor("kern8", [SPC, H, 9, W], F8, kind="ExternalInput")
    x0d = nc.dram_tensor("x0", [SPC, H, W], BF16, kind="ExternalInput")
    xpad = nc.dram_tensor("xpad", [SPC, HP, WP], BF16, kind="ExternalInput")
    ident = nc.dram_tensor("ident", [128, 128], BF16, kind="ExternalInput")
    out = nc.dram_tensor("out", [SPC, H, W], BF16, kind="ExternalOutput")

    xpad_h = xpad[0, 0:1, :].tensor  # underlying handle for raw APs

    with tile.TileContext(nc) as tc:
        with (
            tc.tile_pool(name="ipool", bufs=1) as ipool,
            tc.tile_pool(name="k8pool", bufs=2) as k8pool,
            tc.tile_pool(name="x0pool", bufs=4) as x0pool,
            tc.tile_pool(name="xpool", bufs=4) as xpool,
            tc.tile_pool(name="prpool", bufs=2) as prpool,
            tc.tile_pool(name="pspool", bufs=2, space="PSUM") as pspool,
            tc.tile_pool(name="opool", bufs=4) as opool,
        ):
            it = ipool.tile([128, 128], BF16, tag="ident")
            nc.sync.dma_start(out=it[:, :], in_=ident[:, :])

            def body():
                for b in range(SPC):
                    for r0, p in ROW_BANDS:
                        k8t = k8pool.tile([128, 9, W], F8, tag="k8t")
                        x0t = x0pool.tile([128, W], BF16, tag="x0t")
                        xt = xpool.tile([128, 3, WP], BF16, tag="xt")
                        # all 3 row-shifted xpad views in one DMA
                        # (overlapping AP, rows r0+i+part for i=0..2)
                        nc.scalar.dma_start(
                            out=xt[:p, :, :],
                            in_=AP(
                                xpad_h,
                                b * HP * WP + r0 * WP,
                                [[WP, p], [WP, 3], [1, WP]],
                            ),
                        )
                        nc.scalar.dma_start(
                            out=x0t[:p, :], in_=x0d[b, r0 : r0 + p, :]
                        )
                        nc.sync.dma_start(
                            out=k8t[:p, :, :],
                            in_=kern8[b, r0 : r0 + p, :, :],
                        )

                        prod = prpool.tile([128, 9, W], BF16, tag="prod")
                        ps = pspool.tile([128, W], F32, tag="ps")
                        ot = opool.tile([128, W], BF16, tag="ot")

                        def ksrc(t):
                            return k8t[:p, t, :]

                        def src(t):
                            if t == 4:
                                return x0t[:p, :]
                            i, j = t // 3, t % 3
                            return xt[:p, i, j : j + W]

                        for t in range(9):
                            nc.vector.tensor_tensor(
                                out=prod[:p, t, :], in0=ksrc(t),
                                in1=src(t), op=MULT,
                            )
                            for w0, wc in CHUNKS:
                                nc.tensor.matmul(
                                    out=ps[:p, w0 : w0 + wc],
                                    lhsT=it[:p, :p],
                                    rhs=prod[:p, t, w0 : w0 + wc],
                                    start=(t == 0), stop=(t == 8),
                                )
                        nc.scalar.copy(out=ot[:p, :], in_=ps[:p, :])
                        nc.sync.dma_start(
                            out=out[b, r0 : r0 + p, :], in_=ot[:p, :]
                        )

            if loop_reps == 1:
                body()
            else:
                with tc.For_i(0, loop_reps, 1):
                    body()
    nc.finalize()
    return nc


_NC_CACHE = None


def _get_nc():
    global _NC_CACHE
    if _NC_CACHE is None:
        _NC_CACHE = _build_nc()
    return _NC_CACHE


def _make_in_maps(kernel_arr, input_arr, input0_arr):
    kernel_arr = np.asarray(kernel_arr, dtype=np.float32)
    inp = np.asarray(input_arr, dtype=np.float32)[:, 0]
    inp0 = np.asarray(input0_arr, dtype=np.float32)[:, 0].astype(NP_BF16)

    # [BS, H, 9, W] fp8 planes, row-major by output row
    k8 = np.ascontiguousarray(
        kernel_arr.transpose(0, 2, 1, 3)
    ).astype(NP_F8)

    xp = np.zeros((BS, HP, WP), dtype=NP_BF16)
    xp[:, 1 : H + 1, 1 : W + 1] = inp.astype(NP_BF16)

    ident = np.eye(128, dtype=NP_BF16)

    in_maps = []
    for c in range(NCORES):
        s = slice(c * SPC, (c + 1) * SPC)
        in_maps.append(
            {
                "kern8": np.ascontiguousarray(k8[s]),
                "x0": np.ascontiguousarray(inp0[s]),
                "xpad": np.ascontiguousarray(xp[s]),
                "ident": ident,
            }
        )
    return in_maps


def _run(kernel_arr, input_arr, input0_arr, trace=False):
    in_maps = _make_in_maps(kernel_arr, input_arr, input0_arr)
    nc = _get_nc()
    res = bass_utils.run_bass_kernel_spmd(
        nc, in_maps, list(range(NCORES)), trace=trace
    )
    out = np.concatenate([res.results[c]["out"] for c in range(NCORES)], axis=0)
    out = out.astype(np.float32)
    return np.ascontiguousarray(out.reshape(BS, 1, H, W)), res


def kernel(kernel, input, input0):  # noqa: A002 - names fixed by harness
    out, _ = _run(kernel, input, input0, trace=False)
    return out


# revision 14
# speedup vs baseline: 1.2685x; 1.2685x over previous
"""CSPN 3x3 per-pixel MAC kernel for Trainium2, 8-core data parallel.

out[b,0,h,w] = sum_{t in 0..8, t!=4} K[b,t,h,w] * xpad[b,h+t//3,w+t%3]
             + K[b,4,h,w] * input0[b,0,h,w]

Sharding: batch 16 -> 2 samples per core, pure data parallel.

Numerics (harness rel-err gate is 2e-2; this kernel measures 1.14e-2,
bit-exactly predicted by a host-side pipeline simulation): 6 of 9
kernel planes are float8_e3m4 (4-bit mantissa, range +-15.5 covers the
N(0,1) weights), the other 3 planes + x0 + input stay bf16; products
are bf16, accumulated in f32 PSUM; output bf16, upcast to f32 on host.
fp8 halves the dominant DMA stream again (24.0 -> 18.9 MB/core) at the
cost of the fp8-tap multiplies dropping out of the DVE 2x fast path
(1-byte operands run 1 elem/cycle) -- k=6 fp8 taps balances the DMA
roofline (~59us) against DVE busy (~57us).

Host-side repack: weights stored ROW-MAJOR-BY-OUTPUT-ROW: kern8
[SPC,H,6,W] fp8 (taps 0,2,3,5,6,8) and kern16 [SPC,H,4,W] bf16 (taps
1,4,7 + the x0 row) -- each band's weight data is 2 dma_starts of p
contiguous runs (sequential HBM walk).  A device-side transposed walk
over the natural [9,H,W] layout was measured ~10% slower (partition-
major descriptor order jumps 856KB between planes -> HBM scatter).

Work split across engines (measured: DVE compute and DMA traffic
contend heavily on this silicon -- spreading compute across engines
wins ~20% over DVE-only):
 - DVE: 9 elementwise products per band into a [p,9,W] bf16 tile.
 - PE:  accumulates the 9 product planes into f32 PSUM via
   identity-weight matmuls (out += I.T @ prod_t), 3 PSUM-bank-sized
   column chunks x 9 taps.
 - ACT: copies the f32 PSUM result to a bf16 SBUF tile (DMA cannot
   read PSUM) and issues the x/kern8 DMAs from its HWDGE ring.
 - SP:  issues the kern16 + out DMAs from its HWDGE ring (dedicated
   queues measured better than round-robin).

Negative results measured on this silicon/toolchain, do not retry:
 - GpSimd width-split co-execution: 30% slower, serializes vs DVE.
 - Deeper tile pools (bufs 3): consistently slower than bufs 2.
 - Splitting a band load into partition chunks: much worse (DMA-to-
   SBUF write throughput scales with destination partition count).
 - Partition-base-shifted compute operands: rejected by BIR verifier.
 - bf16 DVE writes to PSUM: rejected by compiler (matmul/memset only).
 - SBUF->SBUF partition-shift copies to single-read x: slower (SBUF
   DMA bytes are the contended resource; the 3x HBM re-read of x via
   one overlapping-AP dma_start is cheaper).
 - SWDGE (Pool-engine) DMA for the kern load: does not execute
   correctly inside hardware loops here.
 - float8_e4m3 weights: 2.7e-2 all-taps / 1.8e-2 4-taps -- over or
   too close to the 2e-2 gate; e3m4 halves the quantization error.
"""

import os
import sys

for _p in ("/opt/trn_rl_repo", "/root/.axon_site/_ro/trn_rl_repo"):
    if os.path.isdir(_p) and _p not in sys.path:
        sys.path.append(_p)

import ml_dtypes
import numpy as np

import concourse.bacc as bacc
import concourse.mybir as mybir
from concourse import bass_utils, tile
from concourse.ap import AP

KS = 3
BS, H, W = 16, 352, 1216
NCORES = 8
SPC = BS // NCORES          # samples per core = 2
HP, WP = H + 2, W + 2       # zero-padded dims: 354 x 1218
BF16 = mybir.dt.bfloat16
F8 = mybir.dt.float8e3
F32 = mybir.dt.float32
NP_BF16 = ml_dtypes.bfloat16
NP_F8 = ml_dtypes.float8_e3m4
MULT = mybir.AluOpType.mult

ROW_BANDS = [(0, 128), (128, 128), (256, 96)]
# PSUM-bank-sized (512 f32) column chunks of W for matmul accumulation
CHUNKS = [(0, 512), (512, 512), (1024, 192)]

F8TAPS = (0, 2, 3, 5, 6, 8)          # planes in kern8, in this order
B16TAPS = (1, 4, 7)                  # planes in kern16; plane 3 = x0
F8IDX = {t: i for i, t in enumerate(F8TAPS)}
B16IDX = {t: i for i, t in enumerate(B16TAPS)}


def _build_nc(loop_reps=1):
    nc = bacc.Bacc(None)
    kern16 = nc.dram_tensor("kern16", [SPC, H, 4, W], BF16, kind="ExternalInput")
    kern8 = nc.dram_tensor("kern8", [SPC, H, 6, W], F8, kind="ExternalInput")
    xpad = nc.dram_tensor("xpad", [SPC, HP, WP], BF16, kind="ExternalInput")
    ident = nc.dram_tensor("ident", [128, 128], BF16, kind="ExternalInput")
    out = nc.dram_tensor("out", [SPC, H, W], BF16, kind="ExternalOutput")

    xpad_h = xpad[0, 0:1, :].tensor  # underlying handle for raw APs

    with tile.TileContext(nc) as tc:
        with (
            tc.tile_pool(name="ipool", bufs=1) as ipool,
            tc.tile_pool(name="k16pool", bufs=2) as k16pool,
            tc.tile_pool(name="k8pool", bufs=2) as k8pool,
            tc.tile_pool(name="xpool", bufs=4) as xpool,
            tc.tile_pool(name="prpool", bufs=2) as prpool,
            tc.tile_pool(name="pspool", bufs=2, space="PSUM") as pspool,
            tc.tile_pool(name="opool", bufs=4) as opool,
        ):
            it = ipool.tile([128, 128], BF16, tag="ident")
            nc.sync.dma_start(out=it[:, :], in_=ident[:, :])

            def body():
                for b in range(SPC):
                    for r0, p in ROW_BANDS:
                        k16t = k16pool.tile([128, 4, W], BF16, tag="k16t")
                        k8t = k8pool.tile([128, 6, W], F8, tag="k8t")
                        xt = xpool.tile([128, 3, WP], BF16, tag="xt")
                        # all 3 row-shifted xpad views in one DMA
                        # (overlapping AP, rows r0+i+part for i=0..2)
                        nc.scalar.dma_start(
                            out=xt[:p, :, :],
                            in_=AP(
                                xpad_h,
                                b * HP * WP + r0 * WP,
                                [[WP, p], [WP, 3], [1, WP]],
                            ),
                        )
                        nc.scalar.dma_start(
                            out=k8t[:p, :, :],
                            in_=kern8[b, r0 : r0 + p, :, :],
                        )
                        nc.sync.dma_start(
                            out=k16t[:p, :, :],
                            in_=kern16[b, r0 : r0 + p, :, :],
                        )

                        prod = prpool.tile([128, 9, W], BF16, tag="prod")
                        ps = pspool.tile([128, W], F32, tag="ps")
                        ot = opool.tile([128, W], BF16, tag="ot")

                        def ksrc(t):
                            if t in F8IDX:
                                return k8t[:p, F8IDX[t], :]
                            return k16t[:p, B16IDX[t], :]

                        def src(t):
                            if t == 4:
                                return k16t[:p, 3, :]
                            i, j = t // 3, t % 3
                            return xt[:p, i, j : j + W]

                        for t in range(9):
                            nc.vector.tensor_tensor(
                                out=prod[:p, t, :], in0=ksrc(t),
                                in1=src(t), op=MULT,
                            )
                            for w0, wc in CHUNKS:
                                nc.tensor.matmul(
                                    out=ps[:p, w0 : w0 + wc],
                                    lhsT=it[:p, :p],
                                    rhs=prod[:p, t, w0 : w0 + wc],
                                    start=(t == 0), stop=(t == 8),
                                )
                        nc.scalar.copy(out=ot[:p, :], in_=ps[:p, :])
                        nc.sync.dma_start(
                            out=out[b, r0 : r0 + p, :], in_=ot[:p, :]
                        )

            if loop_reps == 1:
                body()
            else:
                with tc.For_i(0, loop_reps, 1):
                    body()
    nc.finalize()
    return nc


_NC_CACHE = None


def _get_nc():
    global _NC_CACHE
    if _NC_CACHE is None:
        _NC_CACHE = _build_nc()
    return _NC_CACHE


def _make_in_maps(kernel_arr, input_arr, input0_arr):
    kernel_arr = np.asarray(kernel_arr, dtype=np.float32)
    inp = np.asarray(input_arr, dtype=np.float32)[:, 0]
    inp0 = np.asarray(input0_arr, dtype=np.float32)[:, 0].astype(NP_BF16)

    # [BS, H, 6, W] fp8 planes, row-major by output row
    k8 = np.ascontiguousarray(
        kernel_arr[:, F8TAPS].transpose(0, 2, 1, 3)
    ).astype(NP_F8)
    # [BS, H, 4, W]: bf16 planes + x0 as plane 3
    k16 = np.empty((BS, H, 4, W), dtype=NP_BF16)
    k16[:, :, :3, :] = kernel_arr[:, B16TAPS].transpose(0, 2, 1, 3)
    k16[:, :, 3, :] = inp0

    xp = np.zeros((BS, HP, WP), dtype=NP_BF16)
    xp[:, 1 : H + 1, 1 : W + 1] = inp.astype(NP_BF16)

    ident = np.eye(128, dtype=NP_BF16)

    in_maps = []
    for c in range(NCORES):
        s = slice(c * SPC, (c + 1) * SPC)
        in_maps.append(
            {
                "kern16": np.ascontiguousarray(k16[s]),
                "kern8": np.ascontiguousarray(k8[s]),
                "xpad": np.ascontiguousarray(xp[s]),
                "ident": ident,
            }
        )
    return in_maps


def _run(kernel_arr, input_arr, input0_arr, trace=False):
    in_maps = _make_in_maps(kernel_arr, input_arr, input0_arr)
    nc = _get_nc()
    res = bass_utils.run_bass_kernel_spmd(
        nc, in_maps, list(range(NCORES)), trace=trace
    )
    out = np.concatenate([res.results[c]["out"] for c in range(NCORES)], axis=0)
    out = out.astype(np.float32)
    return np.ascontiguousarray(out.reshape(BS, 1, H, W)), res


def kernel(kernel, input, input0):  # noqa: A002 - names fixed by harness
    out, _ = _run(kernel, input, input0, trace=False)
    return out
